# revision 1
# baseline (speedup 1.0000x reference)
"""GAT-style GNN message passing on 8 TRN2 NeuronCores.

Math: with LEAK=1 the leaky-relu is identity, so
  e[i,j,h] = e_src[i,h] + e_dst[j,h]
and softmax over j cancels e_src (and any row max) exactly:
  attn[i,j,h] = adj[i,j]*exp(e_dst[j,h]) / sum_j adj[i,j]*exp(e_dst[j,h])
  out[i,(h,f)] = (adj @ (z*h))[i,(h,f)] / (adj @ z)[i,h],  z = exp(e_dst)
then elu + log_softmax per row. log_softmax is shift invariant, so
elu(x) is computed as relu(x) + exp(min(x,0)) (drops the uniform -1),
and no max subtraction is needed (y is bounded in [e^-10, ~10]).

Sharding: rows (query nodes) of adj/out across 8 cores. x is row-sharded
too; each core computes its local h slab, all-gathers G=[z*h | z],
then computes its [N/8, 64] output slab locally.

The aggregation matmul adj @ G runs in bf16 at full PE rate but stays
EXACT to ~2^-16: adj entries are 0/1 (exact in bf16) and G is sent as a
bf16 hi/lo split (G = hi + lo, two accumulating matmuls into fp32 PSUM)
— same bytes as fp32, half the PE cycles of the fp32 4-cycle/row mode,
and no bf16->fp32 cast pass over the 4MB adjacency.

All DRAM<->SBUF tensors use partition-major host layouts ([128, ...],
one contiguous run per partition) so each DMA needs ~128 descriptors
(~3.5ns/descriptor on the HWDGE queue otherwise dominates).

Per-core device program (R = N/8 = 512 rows, P=128):
  inputs:  xt [128, KC*R] f32   xt[p, kc*R+r]  = x[c*R+r, kc*128+p]
           wt [128, KC*72] f32  wt[p, kc*72+e] = w_ext[kc*128+p, e]
                                (w_ext = [W | W @ blockdiag-reduced a_dst])
           at [128, NC*R] bf16  at[p, n*R+r]   = adj[c*R+r, n*128+p]
  output:  out_p [128, RC*64]   out_p[p, q*64+f] = out[q*128+p, f]
"""

import sys

import numpy as np

if "/opt/trn_rl_repo" not in sys.path:
    sys.path.insert(0, "/opt/trn_rl_repo")

import ml_dtypes  # noqa: E402

import concourse.bass as bass  # noqa: E402
import concourse.tile as tile  # noqa: E402
from concourse import bacc, mybir  # noqa: E402
from concourse.bass_utils import run_bass_kernel_spmd  # noqa: E402
from concourse.masks import make_identity  # noqa: E402

N_CORES = 8
H = 8
F = 8
HF = H * F  # 64
EXT = HF + H  # 72: [g | z]
K_IN = 1024
P = 128

FP32 = mybir.dt.float32
BF16 = mybir.dt.bfloat16
AFT = mybir.ActivationFunctionType
ALU = mybir.AluOpType


def _bcast_head(ap_ph):
    """[P, H] AP -> [P, H, F] AP broadcasting each head value over F."""
    return bass.AP(
        tensor=ap_ph.tensor,
        offset=ap_ph.offset,
        ap=[ap_ph.ap[0], ap_ph.ap[1], [0, F]],
    )


def build_bass(n_nodes: int) -> bass.Bass:
    R = n_nodes // N_CORES
    KC = K_IN // P  # k-chunks for the h matmul
    NC = n_nodes // P  # j-chunks for the aggregation matmul
    RC = R // P  # 128-row output chunks per core
    assert R % P == 0

    # Bacc (not plain Bass): its finalize() runs move_matmul_waits_to_ldweights
    # + generate_event_semaphores, which legalize multi-wait instructions for
    # walrus (TRN2 allows at most 1 sync wait per instruction).
    nc = bacc.Bacc(num_devices=N_CORES)

    xt = nc.declare_dram_parameter("xt", [P, KC * R], FP32, isOutput=False)
    at = nc.declare_dram_parameter("at", [P, NC * R], BF16, isOutput=False)
    wt = nc.declare_dram_parameter("wt", [P, KC * EXT], FP32, isOutput=False)
    out = nc.declare_dram_parameter("out", [P, RC * HF], FP32, isOutput=True)

    # DRAM collectives concatenate the ranks' buffers FLAT (block-major).
    # G is gathered in two pipelined halves (q-chunks 0..RC/2-1, RC/2..RC-1)
    # so the second AllGather's mesh overlaps the first half's matmuls.
    HB = RC // 2  # q-chunks per half
    g_loc_a = nc.dram_tensor("g_loc_a", [P, HB * 2 * EXT], BF16)
    g_loc_b = nc.dram_tensor("g_loc_b", [P, HB * 2 * EXT], BF16)
    g_full_a = nc.dram_tensor(
        "g_full_a", [N_CORES, P, HB * 2 * EXT], BF16, addr_space="Shared"
    )
    g_full_b = nc.dram_tensor(
        "g_full_b", [N_CORES, P, HB * 2 * EXT], BF16, addr_space="Shared"
    )

    with tile.TileContext(nc) as tc:
        with (
            tc.tile_pool(name="singles", bufs=1) as singles,
            tc.tile_pool(name="bigpsum", bufs=2, space="PSUM") as bigpsum,
            tc.tile_pool(name="smallpsum", bufs=4, space="PSUM") as smallpsum,
            tc.tile_pool(name="work", bufs=4) as work,
            tc.tile_pool(name="post", bufs=4) as post,
        ):
            ident = singles.tile([P, P], FP32)
            make_identity(nc, ident)

            # --- loads (p-major, one run per partition) ---
            w_sb = singles.tile([P, KC, EXT], FP32)
            nc.sync.dma_start(
                out=w_sb, in_=wt[:].rearrange("p (c e) -> p c e", c=KC)
            )
            xt_sb = singles.tile([P, KC, R], FP32)
            xt_view = xt[:].rearrange("p (c r) -> p c r", c=KC)
            nc.sync.dma_start(out=xt_sb[:, : KC // 2, :], in_=xt_view[:, : KC // 2, :])
            nc.sync.dma_start(out=xt_sb[:, KC // 2 :, :], in_=xt_view[:, KC // 2 :, :])

            # --- hT = w_ext.T @ x_loc.T : [EXT, R] (fp32, exact), computed
            # in two column halves so the first half's transposes + AllGather
            # trigger before the second half's matmuls finish. ---
            hT_sb = singles.tile([EXT, R], FP32)
            RH = R // 2
            for half in range(2):
                hT_ps = bigpsum.tile([EXT, RH], FP32, tag="bigps", name=f"hT{half}")
                cols = slice(half * RH, (half + 1) * RH)
                for c in range(KC):
                    nc.tensor.matmul(
                        hT_ps,
                        lhsT=w_sb[:, c, :],
                        rhs=xt_sb[:, c, cols],
                        start=(c == 0),
                        stop=(c == KC - 1),
                    )
                nc.vector.tensor_copy(hT_sb[:, cols], hT_ps)

            # --- per 128-chunk: transpose, z=exp, G=[h*z | z], hi/lo bf16 ---
            ghl_sb = singles.tile([P, RC, 2, EXT], BF16)
            for q in range(RC):
                h_ps = smallpsum.tile([P, EXT], FP32, tag="smallps")
                nc.tensor.transpose(
                    h_ps, hT_sb[:, q * P : (q + 1) * P], ident[:EXT, :EXT]
                )
                g_sb = work.tile([P, EXT], FP32, tag="g")
                z_sb = work.tile([P, H], FP32, tag="z")
                nc.scalar.activation(z_sb, h_ps[:, HF:EXT], AFT.Exp)
                nc.vector.tensor_mul(
                    g_sb[:, 0:HF].rearrange("p (h f) -> p h f", h=H),
                    h_ps[:, 0:HF].rearrange("p (h f) -> p h f", h=H),
                    _bcast_head(z_sb),
                )
                nc.vector.tensor_copy(g_sb[:, HF:EXT], z_sb)
                # hi/lo split: exact bf16 representation of fp32 G
                nc.vector.tensor_copy(ghl_sb[:, q, 0, :], g_sb)
                lo_sb = work.tile([P, EXT], FP32, tag="lo")
                nc.vector.tensor_copy(lo_sb, ghl_sb[:, q, 0, :])
                nc.vector.tensor_sub(lo_sb, g_sb, lo_sb)
                nc.vector.tensor_copy(ghl_sb[:, q, 1, :], lo_sb)
                if q == HB - 1:
                    nc.sync.dma_start(out=g_loc_a[:], in_=ghl_sb[:, :HB])
                    nc.gpsimd.collective_compute(
                        "AllGather",
                        ALU.bypass,
                        replica_groups=[list(range(N_CORES))],
                        ins=[g_loc_a[:]],
                        outs=[g_full_a[:]],
                    )
                elif q == RC - 1:
                    nc.sync.dma_start(out=g_loc_b[:], in_=ghl_sb[:, HB:])
                    nc.gpsimd.collective_compute(
                        "AllGather",
                        ALU.bypass,
                        replica_groups=[list(range(N_CORES))],
                        ins=[g_loc_b[:]],
                        outs=[g_full_b[:]],
                    )

            # --- adjT load (bf16, consumed directly by the PE) ---
            at_sb = singles.tile([P, NC, R], BF16)
            at_view = at[:].rearrange("p (n r) -> p n r", n=NC)
            N_SPLITS = 4
            for s in range(N_SPLITS):
                lo, hi = NC // N_SPLITS * s, NC // N_SPLITS * (s + 1)
                nc.sync.dma_start(out=at_sb[:, lo:hi, :], in_=at_view[:, lo:hi, :])

            # --- load gathered G halves, aggregate: outT += G_n.T @ adjT_n ---
            # g_all_X[p, c, q2, s, e] = (hi,lo)[s] of G[c*R + (q2+off)*128 + p, e]
            g_all_a = singles.tile([P, N_CORES, HB, 2, EXT], BF16)
    
            g_all_b = singles.tile([P, N_CORES, HB, 2, EXT], BF16)
            gfa_view = g_full_a[:].rearrange("c p (q s e) -> p c q s e", q=HB, s=2)
            gfb_view = g_full_b[:].rearrange("c p (q s e) -> p c q s e", q=HB, s=2)
            for s in range(2):
                lo, hi = N_CORES // 2 * s, N_CORES // 2 * (s + 1)
                nc.sync.dma_start(out=g_all_a[:, lo:hi], in_=gfa_view[:, lo:hi])
            for s in range(2):
                lo, hi = N_CORES // 2 * s, N_CORES // 2 * (s + 1)
                nc.sync.dma_start(out=g_all_b[:, lo:hi], in_=gfb_view[:, lo:hi])
            outT_ps = bigpsum.tile([EXT, R], FP32, tag="bigps")
            first = True
            for half, g_all_h, qoff in ((0, g_all_a, 0), (1, g_all_b, HB)):
                for c in range(N_CORES):
                    for q2 in range(HB):
                        n = c * RC + qoff + q2
                        for s in range(2):
                            nc.tensor.matmul(
                                outT_ps,
                                lhsT=g_all_h[:, c, q2, s, :],
                                rhs=at_sb[:, n, :],
                                start=first,
                                stop=(half == 1 and c == N_CORES - 1
                                      and q2 == HB - 1 and s == 1),
                            )
                            first = False
            outT_sb = singles.tile([EXT, R], FP32)
            nc.vector.tensor_copy(outT_sb, outT_ps)

            # --- postprocess, batched per stage across the RC chunks ---
            o_ps = [None] * RC
            for q in range(RC):
                o_ps[q] = smallpsum.tile([P, EXT], FP32, tag="smallps", name=f"o_ps{q}")
                nc.tensor.transpose(
                    o_ps[q], outT_sb[:, q * P : (q + 1) * P], ident[:EXT, :EXT]
                )
            xo = [None] * RC
            for q in range(RC):
                rd = work.tile([P, H], FP32, tag="rd")
                nc.vector.reciprocal(rd, o_ps[q][:, HF:EXT])
                xo[q] = post.tile([P, HF], FP32, tag="xo", name=f"xo{q}")
                nc.vector.tensor_mul(
                    xo[q].rearrange("p (h f) -> p h f", h=H),
                    o_ps[q][:, 0:HF].rearrange("p (h f) -> p h f", h=H),
                    _bcast_head(rd),
                )
            # y = relu(xo) + exp(min(xo, 0))  (= elu + 1; log_softmax shift-safe)
            yo = [None] * RC
            eo = [None] * RC
            for q in range(RC):
                mo = work.tile([P, HF], FP32, tag="mo")
                nc.vector.tensor_scalar_min(mo, xo[q], 0.0)
                eo[q] = post.tile([P, HF], FP32, tag="eo", name=f"eo{q}")
                nc.scalar.activation(eo[q], mo, AFT.Exp)
            for q in range(RC):
                yo[q] = post.tile([P, HF], FP32, tag="yo", name=f"yo{q}")
                nc.vector.scalar_tensor_tensor(
                    out=yo[q], in0=xo[q], scalar=0.0, in1=eo[q],
                    op0=ALU.max, op1=ALU.add,
                )
            # log-softmax over the 64 features (no max subtraction needed:
            # y in (0, ~10], exp stays in fp32 range); batch Exp then Ln to
            # avoid ACT table-set thrash.
            ex = [None] * RC
            sm = [None] * RC
            for q in range(RC):
                ex[q] = post.tile([P, HF], FP32, tag="ex", name=f"ex{q}")
                nc.scalar.activation(ex[q], yo[q], AFT.Exp)
            for q in range(RC):
                sm[q] = post.tile([P, 1], FP32, tag="sm", name=f"sm{q}")
                nc.vector.reduce_sum(sm[q], ex[q], axis=mybir.AxisListType.X)
            out_sb = singles.tile([P, RC, HF], FP32)
            for q in range(RC):
                ls = work.tile([P, 1], FP32, tag="ls")
                nc.scalar.activation(ls, sm[q], AFT.Ln)
                nc.vector.tensor_scalar_sub(out_sb[:, q, :], yo[q], ls)
            nc.sync.dma_start(out=out[:], in_=out_sb)

    # Force all ACT activations (Exp + Ln) onto the one table set containing
    # both, so only ONE ACT_TABLE_LOAD is emitted (early, hidden under DMA)
    # instead of a ~1.3us reload at every Exp<->Ln switch. Set indices must
    # stay aligned with act_info.json, so empty the other sets rather than
    # filtering the list.
    orig_gat = bacc.get_activation_tables

    def _one_set(arch):
        return {
            k: (v if k == "natural_log_exp_and_others" else set())
            for k, v in orig_gat(arch).items()
        }

    bacc.get_activation_tables = _one_set
    try:
        nc.finalize()
    finally:
        bacc.get_activation_tables = orig_gat
    return nc


def _pmajor(a, chunk):
    """[chunk*P, L] -> [P, chunk*L] partition-major layout."""
    n, L = a.shape[0] // P, a.shape[1]
    return np.ascontiguousarray(
        a.reshape(n, P, L).transpose(1, 0, 2).reshape(P, n * L)
    )


def _host_prep(x, adj, W, a_dst, n_nodes):
    """Build per-core input maps."""
    R = n_nodes // N_CORES
    Wd = np.einsum(
        "khf,hf->kh", W.reshape(K_IN, H, F), a_dst, dtype=np.float32
    ).astype(np.float32)
    w_ext = np.concatenate([W, Wd], axis=1).astype(np.float32)  # [1024, 72]
    wt = _pmajor(w_ext, K_IN // P)
    adj_bf = adj.astype(ml_dtypes.bfloat16)  # exact for 0/1
    in_maps = []
    for c in range(N_CORES):
        rows = slice(c * R, (c + 1) * R)
        in_maps.append(
            {
                "xt": _pmajor(np.ascontiguousarray(x[rows].T.astype(np.float32)), K_IN // P),
                "at": _pmajor(np.ascontiguousarray(adj_bf[rows].T), n_nodes // P),
                "wt": wt,
            }
        )
    return in_maps


_BUILT = {}


def run(x, adj, W, a_dst, trace=False):
    n_nodes = x.shape[0]
    R = n_nodes // N_CORES
    RC = R // P
    if n_nodes not in _BUILT:
        _BUILT[n_nodes] = build_bass(n_nodes)
    nc = _BUILT[n_nodes]
    in_maps = _host_prep(x, adj, W, a_dst, n_nodes)
    res = run_bass_kernel_spmd(
        nc, in_maps, list(range(N_CORES)), trace=trace
    )
    blocks = []
    for c in range(N_CORES):
        o = res.results[c]["out"]  # [P, RC*HF] p-major
        blocks.append(
            o.reshape(P, RC, HF).transpose(1, 0, 2).reshape(R, HF)
        )
    return np.concatenate(blocks, axis=0).astype(np.float32), res


def kernel(x, adj, W, a_src, a_dst):
    x = np.asarray(x, dtype=np.float32)
    adj = np.asarray(adj)
    W = np.asarray(W, dtype=np.float32)
    a_dst = np.asarray(a_dst, dtype=np.float32)
    out, _ = run(x, adj, W, a_dst, trace=False)
    return out



# revision 2
# speedup vs baseline: 1.4703x; 1.4703x over previous
"""GAT-style GNN message passing on 8 TRN2 NeuronCores (fp8/DoubleRow version).

Math: with LEAK=1 the leaky-relu is identity, so softmax over j cancels
e_src exactly:
  out[i,(h,f)] = (adj @ (z*h))[i,(h,f)] / (adj @ z)[i,h],  z = exp(h @ a_dst)
then elu + log_softmax per row (elu(x)+1 = relu(x)+exp(min(x,0)); the +1
is a uniform shift which log_softmax cancels).

All matmuls run as fp8e4 DoubleRow (2 contractions/cycle):
 - adj entries are 0/1 (exact in fp8); per-core slab is 2MB instead of 4.
 - x, W are fp8 with power-of-2 column scaling so values sit in e4m3's
   normal range: h' = x @ (16W) = 16h, e' = x @ (64Wd) = 64*e_dst.
   z = exp(e'/64) (ACT scale), g = (h'/16)*z via one scalar_tensor_tensor.
 - G = [h*z | z | pad] fp8; the softmax ratio uses the SAME rounded z in
   numerator and denominator, so z rounding only reweights attention
   (error ~0.06/sqrt(2048)); dominant error is g/h rounding ~ 10% of the
   signal, and |out| ~ 0.04 vs log_softmax scale ~4.6 => rel err ~ 4e-3,
   well under the 2e-2 gate.
 - E is padded 72->80 so the DoubleRow pair stride (80B) is 16B-aligned
   (walrus double_row_stride_alignment).  Pad columns only produce junk
   PSUM rows 72:80 which are never read (pad is zeroed anyway for the
   gathered G so the wire bytes are deterministic).

Cross-core: ONE AllGather of the 40KB G contribution (two AllGathers
serialize in ncfw at ~13us each; one is strictly better). The collective
entry barrier absorbs the inter-core launch skew, so the pre-trigger path
is kept minimal: wt/xt loads + h-matmul + G production only. The 2MB adj
slab streams on the second HWDGE ring (ACT) so it never blocks the
critical path's SP-ring FIFO.

Per-core layouts (R=512 rows/core, P=128, KK=4 k-blocks of 256):
  xt [128, 2*4*2*256] fp8  xt[p, h,kk,s,r] = x[c*R + h*256 + r, kk*256+s*128+p]
  wt [128, 4*2*80]    fp8  wt[p, kk,s,e]   = w_ext[kk*256+s*128+p, e]
  at [128, 16*2*512]  fp8  at[p, blk,s,r]  = adj[c*R + r, blk*256+s*128+p]
  out [128, 4*64]     f32  out[p, q*64+f]  = out[c*R + q*128 + p, f]
"""

import sys

import numpy as np

if "/opt/trn_rl_repo" not in sys.path:
    sys.path.insert(0, "/opt/trn_rl_repo")

import ml_dtypes  # noqa: E402

import concourse.bass as bass  # noqa: E402
import concourse.tile as tile  # noqa: E402
from concourse import bacc, mybir  # noqa: E402
from concourse.bass_utils import run_bass_kernel_spmd  # noqa: E402
from concourse.masks import make_identity  # noqa: E402

N_CORES = 8
H = 8
F = 8
HF = H * F  # 64
EXT = HF + H  # 72: [g | z]
EP = 80  # padded to 16B-aligned DoubleRow pair stride
K_IN = 1024
P = 128
KK = K_IN // 256  # 4 double-row k-blocks for the h matmul

FP32 = mybir.dt.float32
FP8 = mybir.dt.float8e4
AFT = mybir.ActivationFunctionType
ALU = mybir.AluOpType
DR = mybir.MatmulPerfMode.DoubleRow


def _bcast_f(ap_ph, n):
    """[P, H] AP -> [P, H, n] AP broadcasting each head value over n."""
    return bass.AP(
        tensor=ap_ph.tensor,
        offset=ap_ph.offset,
        ap=[ap_ph.ap[0], ap_ph.ap[1], [0, n]],
    )


def build_bass(n_nodes: int) -> bass.Bass:
    R = n_nodes // N_CORES  # 512
    NBLK = n_nodes // 256  # 16 double-row j-blocks for the aggregation
    RC = R // P  # 4 row chunks per core
    BB = RC // 2  # 2 pair-blocks of local G

    nc = bacc.Bacc(num_devices=N_CORES)

    xt = nc.declare_dram_parameter("xt", [P, 2 * KK * 2 * 256], FP8, isOutput=False)
    at = nc.declare_dram_parameter("at", [P, NBLK * 2 * R], FP8, isOutput=False)
    wt = nc.declare_dram_parameter("wt", [P, KK * 2 * EP], FP8, isOutput=False)
    out = nc.declare_dram_parameter("out", [P, RC * HF], FP32, isOutput=True)

    g_loc = nc.dram_tensor("g_loc", [P, BB * 2 * EP], FP8)
    g_full = nc.dram_tensor(
        "g_full", [N_CORES, P, BB * 2 * EP], FP8, addr_space="Shared"
    )

    with tile.TileContext(nc) as tc:
        with (
            tc.tile_pool(name="singles", bufs=1) as singles,
            tc.tile_pool(name="bigpsum", bufs=2, space="PSUM") as bigpsum,
            tc.tile_pool(name="smallpsum", bufs=4, space="PSUM") as smallpsum,
            tc.tile_pool(name="work", bufs=4) as work,
        ):
            ident = singles.tile([P, P], FP32)
            make_identity(nc, ident)

            # --- critical-path loads on the SP ring: wt first, xt halves ---
            w_sb = singles.tile([P, KK, 2, EP], FP8)
            nc.sync.dma_start(
                out=w_sb, in_=wt[:].rearrange("p (k s e) -> p k s e", k=KK, s=2)
            )
            xt_sb = singles.tile([P, 2, KK, 2, 256], FP8)
            xt_view = xt[:].rearrange(
                "p (h k s r) -> p h k s r", h=2, k=KK, s=2
            )
            nc.sync.dma_start(out=xt_sb[:, 0], in_=xt_view[:, 0])
            nc.sync.dma_start(out=xt_sb[:, 1], in_=xt_view[:, 1])

            # --- bulk adjacency on the ACT ring (never blocks SP FIFO) ---
            at_sb = singles.tile([P, NBLK, 2, R], FP8)
            at_view = at[:].rearrange("p (b s r) -> p b s r", b=NBLK, s=2)
            N_SPLITS = 4
            for sp in range(N_SPLITS):
                lo, hi = NBLK // N_SPLITS * sp, NBLK // N_SPLITS * (sp + 1)
                nc.scalar.dma_start(out=at_sb[:, lo:hi], in_=at_view[:, lo:hi])

            # --- G production: h' = (16W|64Wd).T @ x.T in column halves,
            # transpose 128-chunks, z = exp(e'/64), g = (h'/16)*z ---
            g2 = singles.tile([P, BB, 2, EP], FP8)
            # zero the pad columns once (deterministic wire bytes; junk-free)
            g2_base = g2[:, 0, 0, :]
            pad_ap = bass.AP(
                tensor=g2_base.tensor,
                offset=g2_base.offset + EXT,
                ap=[g2_base.ap[0], [EP, BB * 2], [1, EP - EXT]],
            )
            nc.vector.memset(pad_ap, 0.0)

            hT_sb = singles.tile([EXT, 2, 256], FP32)
            for half in range(2):
                hT_ps = bigpsum.tile([EP, 256], FP32, tag="bigps", name=f"hT{half}")
                for k in range(KK):
                    nc.tensor.matmul(
                        hT_ps,
                        lhsT=w_sb[:, k],
                        rhs=xt_sb[:, half, k],
                        start=(k == 0),
                        stop=(k == KK - 1),
                        perf_mode=DR,
                    )
                nc.vector.tensor_copy(hT_sb[:, half], hT_ps[:EXT])
                for qq in range(2):
                    q = half * 2 + qq
                    h_ps = smallpsum.tile([P, EXT], FP32, tag="smallps")
                    nc.tensor.transpose(
                        h_ps,
                        hT_sb[:, half, qq * P : (qq + 1) * P],
                        ident[:EXT, :EXT],
                    )
                    # z (fp8) straight into the G tile
                    zslice = g2[:, q // 2, q % 2, HF:EXT]
                    nc.scalar.activation(
                        zslice, h_ps[:, HF:EXT], AFT.Exp, scale=1.0 / 64.0
                    )
                    # g = (h'/16) * z, reading the rounded fp8 z back
                    nc.vector.scalar_tensor_tensor(
                        out=g2[:, q // 2, q % 2, 0:HF].rearrange(
                            "p (h f) -> p h f", h=H
                        ),
                        in0=h_ps[:, 0:HF].rearrange("p (h f) -> p h f", h=H),
                        scalar=1.0 / 16.0,
                        in1=_bcast_f(zslice, F),
                        op0=ALU.mult,
                        op1=ALU.mult,
                    )
            nc.sync.dma_start(out=g_loc[:], in_=g2)

            # --- ONE AllGather of the 40KB G contribution ---
            nc.gpsimd.collective_compute(
                "AllGather",
                ALU.bypass,
                replica_groups=[list(range(N_CORES))],
                ins=[g_loc[:]],
                outs=[g_full[:]],
            )

            # --- gathered G loads split across both HWDGE rings ---
            g_all = singles.tile([P, N_CORES, BB, 2, EP], FP8)
            gf_view = g_full[:].rearrange("c p (b s e) -> p c b s e", b=BB, s=2)
            ring = [nc.sync, nc.scalar, nc.sync, nc.scalar]
            for grp in range(4):
                lo, hi = grp * 2, grp * 2 + 2
                ring[grp].dma_start(out=g_all[:, lo:hi], in_=gf_view[:, lo:hi])

            # --- aggregation: outT += G_blk.T @ adjT_blk (16 DR matmuls) ---
            outT_ps = bigpsum.tile([EP, R], FP32, tag="bigps", name="outT")
            i = 0
            for c in range(N_CORES):
                for b in range(BB):
                    nc.tensor.matmul(
                        outT_ps,
                        lhsT=g_all[:, c, b],
                        rhs=at_sb[:, c * BB + b],
                        start=(i == 0),
                        stop=(i == N_CORES * BB - 1),
                        perf_mode=DR,
                    )
                    i += 1
            outT_sb = singles.tile([EXT, R], FP32)
            nc.vector.tensor_copy(outT_sb, outT_ps[:EXT])

            # --- postprocess: transpose chunks, xo = num/den, then batched
            # elu(+1) and log_softmax over the 64 features ---
            xo = singles.tile([P, RC, HF], FP32)
            for q in range(RC):
                o_ps = smallpsum.tile([P, EXT], FP32, tag="smallps")
                nc.tensor.transpose(
                    o_ps, outT_sb[:, q * P : (q + 1) * P], ident[:EXT, :EXT]
                )
                rd = work.tile([P, H], FP32, tag="rd")
                nc.vector.reciprocal(rd, o_ps[:, HF:EXT])
                nc.vector.tensor_mul(
                    xo[:, q].rearrange("p (h f) -> p h f", h=H),
                    o_ps[:, 0:HF].rearrange("p (h f) -> p h f", h=H),
                    _bcast_f(rd[:], F),
                )
            xo_flat = xo[:].rearrange("p q f -> p (q f)")
            mo = work.tile([P, RC * HF], FP32, tag="mo")
            nc.vector.tensor_scalar_min(mo, xo_flat, 0.0)
            eo = work.tile([P, RC * HF], FP32, tag="eo")
            nc.scalar.activation(eo, mo, AFT.Exp)
            yo = singles.tile([P, RC, HF], FP32)
            nc.vector.scalar_tensor_tensor(
                out=yo[:].rearrange("p q f -> p (q f)"),
                in0=xo_flat,
                scalar=0.0,
                in1=eo,
                op0=ALU.max,
                op1=ALU.add,
            )
            ex = work.tile([P, RC, HF], FP32, tag="ex")
            nc.scalar.activation(
                ex[:].rearrange("p q f -> p (q f)"),
                yo[:].rearrange("p q f -> p (q f)"),
                AFT.Exp,
            )
            sm = work.tile([P, RC], FP32, tag="sm")
            nc.vector.reduce_sum(sm, ex, axis=mybir.AxisListType.X)
            ls = work.tile([P, RC], FP32, tag="ls")
            nc.scalar.activation(ls, sm, AFT.Ln)
            out_sb = singles.tile([P, RC, HF], FP32)
            ls_base = ls[:]
            ls_bcast = bass.AP(
                tensor=ls_base.tensor,
                offset=ls_base.offset,
                ap=[ls_base.ap[0], ls_base.ap[1], [0, HF]],
            )
            nc.vector.tensor_sub(out_sb, yo, ls_bcast)
            nc.sync.dma_start(out=out[:], in_=out_sb)

    # Pin all ACT activations (Exp + Ln) onto the single table set holding
    # both so only one ACT_TABLE_LOAD is emitted.
    orig_gat = bacc.get_activation_tables

    def _one_set(arch):
        return {
            k: (v if k == "natural_log_exp_and_others" else set())
            for k, v in orig_gat(arch).items()
        }

    bacc.get_activation_tables = _one_set
    try:
        nc.finalize()
    finally:
        bacc.get_activation_tables = orig_gat
    return nc


def _host_prep(x, adj, W, a_dst, n_nodes):
    """Build per-core input maps (fp8 DoubleRow layouts)."""
    R = n_nodes // N_CORES
    NBLK = n_nodes // 256
    f8 = ml_dtypes.float8_e4m3
    Wd = np.einsum(
        "khf,hf->kh", W.reshape(K_IN, H, F), a_dst, dtype=np.float32
    ).astype(np.float32)
    w_ext = np.zeros((K_IN, EP), dtype=np.float32)
    w_ext[:, :HF] = W * 16.0
    w_ext[:, HF:EXT] = Wd * 64.0
    # wt[p, kk, s, e] = w_ext[kk*256+s*128+p, e]
    wt = np.ascontiguousarray(
        w_ext.reshape(KK, 2, P, EP).transpose(2, 0, 1, 3).reshape(P, KK * 2 * EP)
    ).astype(f8)
    adj_f8 = adj.astype(np.int8).astype(f8)  # exact for 0/1
    x_f8 = x.astype(f8)
    in_maps = []
    for c in range(N_CORES):
        rows = slice(c * R, (c + 1) * R)
        # xt[p, h, kk, s, r] = x[c*R + h*256 + r, kk*256 + s*128 + p]
        xc = x_f8[rows]  # [512, 1024]
        xt = np.ascontiguousarray(
            xc.reshape(2, 256, KK, 2, P)
            .transpose(4, 0, 2, 3, 1)
            .reshape(P, 2 * KK * 2 * 256)
        )
        # at[p, blk, s, r] = adj[c*R + r, blk*256 + s*128 + p]
        ac = adj_f8[rows]  # [512, 4096]
        at = np.ascontiguousarray(
            ac.reshape(R, NBLK, 2, P).transpose(3, 1, 2, 0).reshape(P, NBLK * 2 * R)
        )
        in_maps.append({"xt": xt, "at": at, "wt": wt})
    return in_maps


_BUILT = {}


def run(x, adj, W, a_dst, trace=False):
    n_nodes = x.shape[0]
    R = n_nodes // N_CORES
    RC = R // P
    if n_nodes not in _BUILT:
        _BUILT[n_nodes] = build_bass(n_nodes)
    nc = _BUILT[n_nodes]
    in_maps = _host_prep(x, adj, W, a_dst, n_nodes)
    res = run_bass_kernel_spmd(nc, in_maps, list(range(N_CORES)), trace=trace)
    blocks = []
    for c in range(N_CORES):
        o = res.results[c]["out"]  # [P, RC*HF] p-major
        blocks.append(o.reshape(P, RC, HF).transpose(1, 0, 2).reshape(R, HF))
    return np.concatenate(blocks, axis=0).astype(np.float32), res


def kernel(x, adj, W, a_src, a_dst):
    x = np.asarray(x, dtype=np.float32)
    adj = np.asarray(adj)
    W = np.asarray(W, dtype=np.float32)
    a_dst = np.asarray(a_dst, dtype=np.float32)
    out, _ = run(x, adj, W, a_dst, trace=False)
    return out
